# revision 79
# baseline (speedup 1.0000x reference)
"""Masked multi-head self-attention on 8 trn2 NeuronCores.

Sharding: data-parallel over B (=2) x tensor-parallel over heads (16 -> 4
groups of 4). Core c handles batch c//4, head group c%4. Each core computes
its 4 heads end-to-end plus its partial output projection; the host sums the
4 partials per batch element (the "all-reduce") and adds b_out.

All matmul operands are bf16 (f32 PSUM accumulation). x arrives
host-pretransposed and fused with the qkv weights into one DRAM tensor so
QKV projection needs no on-chip transposes or staging copies and the
startup path is 8 DMAs. Scores for diagonal tiles restrict the matmul/exp
to the causally valid q-range; gpsimd affine_select zero-fills only the
128-wide triangle blocks. AV matmuls are column-restricted with per-region
start/stop. Softmax denominators ride along as a [V|1] ones column (AV psum
row 64); reciprocal + partition_broadcast + one DVE multiply normalize
straight out of PSUM.

Schedule: the attention ki-loop is software-pipelined (AV trails
scores/exp/select by 8 tiles) and QKV blocks for the next t-chunk plus the
deferred output projection of earlier q-chunks are injected between
attention matmuls, keeping the tensor engine busy through exp latency. The
final q-chunk finalizes its last head-pair via a 1-row PE-matmul broadcast
into PSUM with ACT-staged AV copies, landing in partition-0 tmp tiles that
the final (split-K) projection consumes directly, so no partition-shift DMA
sits on the critical tail.
"""

import math

import numpy as np
import ml_dtypes

import concourse.bacc as bacc
import concourse.mybir as mybir
from concourse.tile import TileContext
from concourse.bass_utils import run_bass_kernel_spmd

T, C, H, D = 2048, 1024, 16, 64
NCORES = 8
HPC = 4  # heads per core
GO = 3 * HPC * D  # 768 qkv rows per core
TQ = 512
NQ = T // TQ  # 4
KC = 128
NK = T // KC  # 16
NCC = C // KC  # 8
F32 = mybir.dt.float32
BF16 = mybir.dt.bfloat16
FP8E4 = mybir.dt.float8e4
FP8E5 = mybir.dt.float8e5
DR = mybir.MatmulPerfMode.DoubleRow
NEG = -1.0e30

_CACHED_NC = None


def _build():
    nc = bacc.Bacc("TRN2", target_bir_lowering=False, debug=False, num_devices=NCORES)
    xT_d = nc.dram_tensor("xT", [C, T], BF16, kind="ExternalInput")
    wx_d = nc.dram_tensor("wx", [C, GO + TQ], BF16, kind="ExternalInput")
    woutT_d = nc.dram_tensor("woutT", [HPC * D, C], BF16, kind="ExternalInput")
    bqk_d = nc.dram_tensor("bqk", [128, 4], F32, kind="ExternalInput")
    bvb_d = nc.dram_tensor("bvb", [128, HPC * D], F32, kind="ExternalInput")
    pad_d = nc.dram_tensor("pad", [128, NK], F32, kind="ExternalInput")
    y_d = nc.dram_tensor("y", [T, C], BF16, kind="ExternalOutput")

    AF = mybir.ActivationFunctionType
    ALU = mybir.AluOpType

    with TileContext(nc) as tc:
        with (
            tc.tile_pool(name="const", bufs=1) as constp,
            tc.tile_pool(name="weights", bufs=1) as wp,
            tc.tile_pool(name="xin", bufs=1) as xp,
            tc.tile_pool(name="qk", bufs=1) as qkp,
            tc.tile_pool(name="vst", bufs=1) as vp,
            tc.tile_pool(name="pt", bufs=10) as ptp,
            tc.tile_pool(name="outT", bufs=1) as otp,
            tc.tile_pool(name="ystage", bufs=6) as ysp,
            tc.tile_pool(name="scps", bufs=2, space="PSUM") as scps,
            tc.tile_pool(name="avps", bufs=2, space="PSUM") as avps,
            tc.tile_pool(name="bcps", bufs=2, space="PSUM") as bcps,
        ):
            # --- weights + x chunk-0 arrive fused (one DMA per c-chunk) ---
            wxt = [
                wp.tile([128, GO + TQ], BF16, tag=f"wx{cc}", name=f"wx{cc}")
                for cc in range(NCC)
            ]
            xr = [
                xp.tile([128, T - TQ], BF16, tag=f"xr_{cc}", name=f"xr_{cc}")
                for cc in range(NCC)
            ]
            wq = [wxt[cc][:, 0:GO] for cc in range(NCC)]

            def xap(tch, cc):
                if tch == 0:
                    return wxt[cc][:, GO : GO + TQ]
                return xr[cc][:, (tch - 1) * TQ : tch * TQ]

            for cc in range(NCC):
                nc.sync.dma_start(
                    wxt[cc][:], wx_d[cc * 128 : (cc + 1) * 128, :]
                )

            # ---------------- constants ----------------
            bqk = constp.tile([128, 4], F32, tag="bqk")
            nc.sync.dma_start(bqk[:], bqk_d[:, :])
            bvb = constp.tile([128, HPC * D], F32, tag="bvb")
            nc.sync.dma_start(bvb[:], bvb_d[:, :])
            pad = constp.tile([128, NK], F32, tag="pad")
            nc.sync.dma_start(pad[:], pad_d[:, :])
            ones4 = constp.tile([128, HPC], BF16, tag="ones4")
            nc.vector.memset(ones4[:], 1.0)
            ones_row = constp.tile([65, 64], BF16, tag="ones_row")
            nc.vector.memset(ones_row[:], 1.0)
            recl_sb = [
                [
                    constp.tile([65, TQ], BF16, tag=f"recl{i}{h}", name=f"recl{i}{h}")
                    for h in range(2)
                ]
                for i in range(2)
            ]
            rec_sb = [
                [
                    constp.tile([65, TQ], F32, tag=f"rec{i}{h}", name=f"rec{i}{h}")
                    for h in range(2)
                ]
                for i in range(2)
            ]

            wo = []
            for j in range(2):
                w = wp.tile([128, C], BF16, tag=f"wo{j}", name=f"wo{j}")
                nc.sync.dma_start(w[:], woutT_d[j * 128 : (j + 1) * 128, :])
                wo.append(w)
            for cc in range(NCC):
                nc.sync.dma_start(
                    xr[cc][:], xT_d[cc * 128 : (cc + 1) * 128, TQ:T]
                )
            wo_odd = []
            for p in range(2):
                w = wp.tile([64, C], BF16, tag=f"wo_odd{p}")
                nc.sync.dma_start(
                    w[:], woutT_d[p * 128 + 64 : p * 128 + 128, :]
                )
                wo_odd.append(w)
            tmp_last = [
                [
                    otp.tile([64, TQ], BF16, tag=f"tl{p}{h}", name=f"tl{p}{h}")
                    for h in range(2)
                ]
                for p in range(2)
            ]

            # ---------------- static activation storage ----------------
            qt = [
                [
                    qkp.tile([128, TQ], BF16, tag=f"qt{p}_{i}", name=f"qt{p}_{i}")
                    for i in range(NQ)
                ]
                for p in range(2)
            ]
            kt = [
                [
                    qkp.tile([128, TQ], BF16, tag=f"kt{p}_{i}", name=f"kt{p}_{i}")
                    for i in range(NQ)
                ]
                for p in range(2)
            ]
            vt = [
                vp.tile([128, HPC, D + 1], BF16, tag=f"v{k}", name=f"v{k}")
                for k in range(NK)
            ]
            outT = [
                [
                    otp.tile([128, TQ], BF16, tag=f"o{p}_{q}", name=f"o{p}_{q}")
                    for q in range(NQ)
                ]
                for p in range(2)
            ]

            # ------- background PE blocks (QKV of next chunk, deferred proj) -
            def qkv_blocks(tch):
                blocks = []

                def mk_a(ot):
                    def blk():
                        pa = bcps.tile(
                            [128, TQ], F32, tag="bcyp", name=f"pa{tch}_{ot}"
                        )
                        for cc in range(NCC):
                            nc.tensor.matmul(
                                pa[:],
                                wq[cc][:, ot * 128 : (ot + 1) * 128],
                                xap(tch, cc),
                                start=(cc == 0),
                                stop=(cc == NCC - 1),
                            )
                        dst = qt[ot][tch] if ot < 2 else kt[ot - 2][tch]
                        nc.vector.tensor_scalar_add(
                            dst[:], pa[:], bqk[:, ot : ot + 1]
                        )

                    return blk

                def mk_b(tt):
                    def blk():
                        pb = bcps.tile(
                            [128, HPC * D], F32, tag="bcyp", name=f"pb{tch}_{tt}"
                        )
                        for cc in range(NCC):
                            nc.tensor.matmul(
                                pb[:],
                                xap(tch, cc)[:, tt * 128 : (tt + 1) * 128],
                                wq[cc][:, 2 * HPC * D : 3 * HPC * D],
                                start=(cc == 0),
                                stop=(cc == NCC - 1),
                            )
                        k_id = tch * 4 + tt
                        nc.vector.tensor_add(
                            vt[k_id][:, :, 0:D],
                            pb[:].rearrange("p (h d) -> p h d", d=D),
                            bvb[:].rearrange("p (h d) -> p h d", d=D),
                        )
                        nc.vector.tensor_copy(vt[k_id][:, :, D], ones4[:])

                    return blk

                for ot in range(4):
                    blocks.append(mk_a(ot))
                for tt in range(4):
                    blocks.append(mk_b(tt))
                return blocks

            def proj_blocks(qc, split_dma=False):
                blocks = []

                def mk(tt):
                    def blk():
                        t0 = qc * TQ + tt * 128
                        ys = ysp.tile(
                            [128, C], BF16, tag="ys", name=f"ys{qc}{tt}"
                        )
                        for oc in range(2):
                            yp = bcps.tile(
                                [128, TQ], F32, tag="bcyp", name=f"yp{qc}{tt}{oc}"
                            )
                            nc.tensor.matmul(
                                yp[:],
                                outT[0][qc][:, tt * 128 : (tt + 1) * 128],
                                wo[0][:, oc * TQ : (oc + 1) * TQ],
                                start=True,
                                stop=False,
                            )
                            nc.tensor.matmul(
                                yp[:],
                                outT[1][qc][:, tt * 128 : (tt + 1) * 128],
                                wo[1][:, oc * TQ : (oc + 1) * TQ],
                                start=False,
                                stop=True,
                            )
                            nc.vector.tensor_copy(
                                ys[:, oc * TQ : (oc + 1) * TQ], yp[:]
                            )
                            if split_dma:
                                nc.sync.dma_start(
                                    y_d[t0 : t0 + 128, oc * TQ : (oc + 1) * TQ],
                                    ys[:, oc * TQ : (oc + 1) * TQ],
                                )
                        if not split_dma:
                            nc.sync.dma_start(y_d[t0 : t0 + 128, :], ys[:])

                    return blk

                for tt in range(4):
                    blocks.append(mk(tt))
                return blocks

            # ---------------- main schedule ----------------
            for blk in qkv_blocks(0):
                blk()

            for tch in range(NQ):
                qc = tch
                nk = (qc + 1) * 4

                bg = []
                if tch + 1 < NQ:
                    bg += qkv_blocks(tch + 1)
                # spread proj(qc-1) across the next two attention chunks
                if tch == 1:
                    bg += proj_blocks(0)[:2]
                elif tch == 2:
                    bg += proj_blocks(0)[2:] + proj_blocks(1)[:2]
                elif tch == 3:
                    bg += proj_blocks(1)[2:] + proj_blocks(2)
                nbg = len(bg)
                slots = 2 * nk
                emitted = 0
                slot = 0

                def inject():
                    nonlocal emitted
                    target = min(nbg, math.ceil(nbg * slot / slots))
                    while emitted < target:
                        bg[emitted]()
                        emitted += 1

                for p in ((1, 0) if qc >= NQ - 2 else (0, 1)):
                    av_e = avps.tile([65, TQ], F32, tag="av", name=f"ave{p}{qc}")
                    av_o = avps.tile([65, TQ], F32, tag="av", name=f"avo{p}{qc}")
                    pend = []  # (ki, pt) awaiting AV matmuls

                    def emit_av(ki, pt):
                        diag_j = ki - qc * 4 if ki >= qc * 4 else None
                        if diag_j is None:
                            regions = [(0, TQ, ki == 0, False)]
                        else:
                            v0 = 128 * diag_j
                            st = ki == 0
                            regions = [(v0, v0 + 128, st, True)]
                            if v0 + 128 < TQ:
                                regions.append((v0 + 128, TQ, st, False))
                        for h, av in ((0, av_e), (1, av_o)):
                            for c0, c1, st, sp in regions:
                                nc.tensor.matmul(
                                    av[:, c0:c1],
                                    vt[ki][:, 2 * p + h, :],
                                    pt[:, h, c0:c1],
                                    start=st,
                                    stop=sp,
                                    skip_group_check=True,
                                )

                    for ki in range(nk):
                        kch, kof = ki // 4, (ki % 4) * 128
                        diag_j = ki - qc * 4 if ki >= qc * 4 else None
                        vo = 128 * diag_j if diag_j is not None else 0
                        sc = scps.tile(
                            [128, 2 * TQ], F32, tag="sc", name=f"s{p}{qc}{ki}"
                        )
                        nc.tensor.matmul(
                            sc[:, vo:TQ],
                            kt[p][kch][0:64, kof : kof + KC],
                            qt[p][qc][0:64, vo:TQ],
                            start=True,
                            stop=True,
                        )
                        nc.tensor.matmul(
                            sc[:, TQ + vo : 2 * TQ],
                            kt[p][kch][64:128, kof : kof + KC],
                            qt[p][qc][64:128, vo:TQ],
                            start=True,
                            stop=True,
                        )
                        pt = ptp.tile([128, 2, TQ], BF16, tag="pt")
                        sc3 = sc[:].rearrange("p (h q) -> p h q", h=2)
                        if diag_j is None or diag_j == 0:
                            nc.scalar.activation(
                                pt[:], sc3[:], AF.Exp,
                                bias=pad[:, ki : ki + 1], scale=1.0,
                            )
                        else:
                            vw = TQ - vo
                            nc.scalar.activation(
                                pt[:, :, vo : vo + vw],
                                sc3[:, :, vo : vo + vw],
                                AF.Exp,
                                bias=pad[:, ki : ki + 1],
                                scale=1.0,
                            )
                        if diag_j is not None:
                            j = diag_j
                            q0 = 128 * j
                            for hh in range(2):
                                nc.gpsimd.affine_select(
                                    out=pt[:, hh, q0 : q0 + 128],
                                    in_=pt[:, hh, q0 : q0 + 128],
                                    compare_op=ALU.is_ge,
                                    fill=0.0,
                                    base=0,
                                    pattern=[[1, 128]],
                                    channel_multiplier=-1,
                                )
                        if len(pend) == 8:
                            emit_av(*pend.pop(0))
                            slot += 1
                            inject()
                        pend.append((ki, pt))
                    while pend:
                        emit_av(*pend.pop(0))
                        slot += 1
                        inject()

                    # finalize both heads with overlapped chains:
                    # recips first, then shift-DMAs, broadcasts, multiplies
                    # (odd head first -- its extra outT shift DMA is longest)
                    heads = ((1, av_o), (0, av_e))
                    if qc == NQ - 1 and p == 0:
                        # tail path: PE-matmul broadcast + staged AV copy,
                        # results land in partition-0-based tmp tiles that the
                        # final projection consumes directly (no shift DMA).
                        # The truly-last pair (p==0) stages the AV copy on the
                        # by-then-idle ACT engine; pair 1 uses DVE so it does
                        # not delay pair 0's exps queued on ACT.
                        for h, av in heads:
                            with nc.allow_low_precision(reason="softmax recip"):
                                nc.vector.reciprocal(
                                    recl_sb[p][h][64:65, :], av[64:65, :]
                                )
                        bcl = {}
                        for h, av in heads:
                            bc_ps = bcps.tile(
                                [64, TQ], F32, tag="bcyp", name=f"bcl{p}{h}"
                            )
                            nc.tensor.matmul(
                                bc_ps[:],
                                ones_row[64:65, :],
                                recl_sb[p][h][64:65, :],
                                start=True,
                                stop=True,
                            )
                            bcl[h] = bc_ps
                        avl = {}
                        for h, av in heads:
                            a = ysp.tile(
                                [64, TQ], BF16, tag="avsl", name=f"avsl{p}{h}"
                            )
                            if p == 0:
                                nc.scalar.copy(a[:], av[0:64, :])
                            else:
                                nc.vector.tensor_copy(a[:], av[0:64, :])
                            avl[h] = a
                        for h, av in heads:
                            nc.vector.tensor_mul(
                                tmp_last[p][h][:], avl[h][:], bcl[h][:]
                            )
                        continue
                    for h, av in heads:
                        with nc.allow_low_precision(reason="softmax recip"):
                            nc.vector.reciprocal(
                                rec_sb[p][h][64:65, :], av[64:65, :]
                            )
                    recbs = {}
                    for h, av in heads:
                        recb = ysp.tile(
                            [1, TQ], F32, tag="recb", name=f"recb{p}{qc}{h}"
                        )
                        nc.sync.dma_start(recb[:], rec_sb[p][h][64:65, :])
                        recbs[h] = recb
                    bcs = {}
                    for h, av in heads:
                        bc_sb = ysp.tile(
                            [64, TQ], F32, tag="bcsb", name=f"bcs{p}{qc}{h}"
                        )
                        nc.gpsimd.partition_broadcast(
                            bc_sb[:], recbs[h][:], channels=64
                        )
                        bcs[h] = bc_sb
                    for h, av in heads:
                        if h == 0:
                            nc.vector.tensor_mul(
                                outT[p][qc][0:64, :], av[0:64, :], bcs[h][:]
                            )
                        else:
                            tmp_o = ysp.tile(
                                [64, TQ], BF16, tag="tmpo", name=f"tmpo{p}{qc}"
                            )
                            nc.vector.tensor_mul(
                                tmp_o[:], av[0:64, :], bcs[h][:]
                            )
                            nc.sync.dma_start(
                                outT[p][qc][64:128, :], tmp_o[:]
                            )

                while emitted < nbg:
                    bg[emitted]()
                    emitted += 1

            # final projection: pair-1 outT plus the last pair's
            # partition-0 tmp tiles (split-K), psum->sbuf copies split
            # across DVE and ACT, per-half y DMAs
            qf = NQ - 1
            for tt in range(4):
                t0 = qf * TQ + tt * 128
                ys = ysp.tile([128, C], BF16, tag="ys", name=f"ysf{tt}")
                for oc in range(2):
                    yp = bcps.tile(
                        [128, TQ], F32, tag="bcyp", name=f"ypf{tt}{oc}"
                    )
                    nc.tensor.matmul(
                        yp[:],
                        outT[1][qf][:, tt * 128 : (tt + 1) * 128],
                        wo[1][:, oc * TQ : (oc + 1) * TQ],
                        start=True,
                        stop=False,
                    )
                    for i, hh in enumerate((0, 1)):
                        lhsT = tmp_last[0][hh][:, tt * 128 : (tt + 1) * 128]
                        rhs = (
                            wo[0][0:64, oc * TQ : (oc + 1) * TQ]
                            if hh == 0
                            else wo_odd[0][:, oc * TQ : (oc + 1) * TQ]
                        )
                        nc.tensor.matmul(
                            yp[:], lhsT, rhs,
                            start=False, stop=(i == 1),
                        )
                    if oc == 0:
                        nc.vector.tensor_copy(
                            ys[:, oc * TQ : (oc + 1) * TQ], yp[:]
                        )
                    else:
                        nc.scalar.copy(
                            ys[:, oc * TQ : (oc + 1) * TQ], yp[:]
                        )
                    nc.sync.dma_start(
                        y_d[t0 : t0 + 128, oc * TQ : (oc + 1) * TQ],
                        ys[:, oc * TQ : (oc + 1) * TQ],
                    )

    nc.compile()
    return nc


def _get_nc():
    global _CACHED_NC
    if _CACHED_NC is None:
        _CACHED_NC = _build()
    return _CACHED_NC


def _make_in_maps(x, attention_mask, W_qkv, b_qkv, W_out, b_out):
    bf16 = ml_dtypes.bfloat16
    x = np.asarray(x, dtype=np.float32)
    attention_mask = np.asarray(attention_mask, dtype=np.float32)
    W_qkv = np.asarray(W_qkv, dtype=np.float32)
    b_qkv = np.asarray(b_qkv, dtype=np.float32)
    W_out = np.asarray(W_out, dtype=np.float32)

    in_maps = []
    for core in range(NCORES):
        b = core // 4
        g = core % 4
        s = g * HPC * D
        e = (g + 1) * HPC * D
        Wq = W_qkv[s:e] * 0.125
        Wk = W_qkv[C + s : C + e]
        Wv = W_qkv[2 * C + s : 2 * C + e]
        xTb = x[b].T.astype(bf16)
        wqkvT = np.concatenate([Wq, Wk, Wv], axis=0).T.astype(bf16)
        wx = np.ascontiguousarray(np.concatenate([wqkvT, xTb[:, 0:TQ]], axis=1))
        woutT = np.ascontiguousarray(W_out[:, s:e].T.astype(bf16))
        bq = b_qkv[s:e] * 0.125
        bk = b_qkv[C + s : C + e]
        bqk = np.ascontiguousarray(
            np.stack([bq[0:128], bq[128:256], bk[0:128], bk[128:256]], axis=1)
        )
        bv = b_qkv[2 * C + s : 2 * C + e]
        bvb = np.ascontiguousarray(np.broadcast_to(bv, (128, HPC * D)))
        padv = np.ascontiguousarray(
            ((1.0 - attention_mask[b]) * NEG).reshape(NK, 128).T
        )
        in_maps.append(
            {
                "xT": np.ascontiguousarray(xTb),
                "wx": wx,
                "woutT": woutT,
                "bqk": bqk,
                "bvb": bvb,
                "pad": padv,
            }
        )
    return in_maps


def kernel(x, attention_mask, W_qkv, b_qkv, W_out, b_out, _trace=False):
    nc = _get_nc()
    in_maps = _make_in_maps(x, attention_mask, W_qkv, b_qkv, W_out, b_out)
    res = run_bass_kernel_spmd(
        nc, in_maps, core_ids=list(range(NCORES)), trace=_trace
    )
    B = np.asarray(x).shape[0]
    y = np.zeros((B, T, C), dtype=np.float32)
    for b in range(B):
        acc = res.results[4 * b]["y"].astype(np.float32)
        for g in range(1, 4):
            acc = acc + res.results[4 * b + g]["y"].astype(np.float32)
        y[b] = acc
    y += np.asarray(b_out, dtype=np.float32)
    if _trace:
        kernel._last_results = res
    return y


# revision 97
# speedup vs baseline: 1.0027x; 1.0027x over previous
"""Masked multi-head self-attention on 8 trn2 NeuronCores.

Sharding: data-parallel over B (=2) x tensor-parallel over heads (16 -> 4
groups of 4). Core c handles batch c//4, head group c%4. Each core computes
its 4 heads end-to-end plus its partial output projection; the host sums the
4 partials per batch element (the "all-reduce") and adds b_out.

All matmul operands are bf16 (f32 PSUM accumulation). x arrives
host-pretransposed and fused with the qkv weights into one DRAM tensor so
QKV projection needs no on-chip transposes or staging copies and the
startup path is 8 DMAs. Scores for diagonal tiles restrict the matmul/exp
to the causally valid q-range; gpsimd affine_select zero-fills only the
128-wide triangle blocks. AV matmuls are column-restricted with per-region
start/stop. Softmax denominators ride along as a [V|1] ones column (AV psum
row 64); reciprocal + partition_broadcast + one DVE multiply normalize
straight out of PSUM.

Schedule: the attention ki-loop is software-pipelined (AV trails
scores/exp/select by 8 tiles) and QKV blocks for the next t-chunk plus the
deferred output projection of earlier q-chunks are injected between
attention matmuls, keeping the tensor engine busy through exp latency. The
final q-chunk finalizes its last head-pair via a 1-row PE-matmul broadcast
into PSUM with ACT-staged AV copies, landing in partition-0 tmp tiles that
the final (split-K) projection consumes directly, so no partition-shift DMA
sits on the critical tail.
"""

import math

import numpy as np
import ml_dtypes

import concourse.bacc as bacc
import concourse.mybir as mybir
from concourse.tile import TileContext
from concourse.bass_utils import run_bass_kernel_spmd

T, C, H, D = 2048, 1024, 16, 64
NCORES = 8
HPC = 4  # heads per core
GO = 3 * HPC * D  # 768 qkv rows per core
TQ = 512
NQ = T // TQ  # 4
KC = 128
NK = T // KC  # 16
NCC = C // KC  # 8
F32 = mybir.dt.float32
BF16 = mybir.dt.bfloat16
FP8E4 = mybir.dt.float8e4
FP8E5 = mybir.dt.float8e5
DR = mybir.MatmulPerfMode.DoubleRow
NEG = -1.0e30

_CACHED_NC = None


def _build():
    nc = bacc.Bacc("TRN2", target_bir_lowering=False, debug=False, num_devices=NCORES)
    xT_d = nc.dram_tensor("xT", [C, T], BF16, kind="ExternalInput")
    wx_d = nc.dram_tensor("wx", [C, GO + TQ], BF16, kind="ExternalInput")
    woutT_d = nc.dram_tensor("woutT", [HPC * D, C], BF16, kind="ExternalInput")
    bqk_d = nc.dram_tensor("bqk", [128, 4], F32, kind="ExternalInput")
    bvb_d = nc.dram_tensor("bvb", [128, HPC * D], F32, kind="ExternalInput")
    pad_d = nc.dram_tensor("pad", [128, NK], F32, kind="ExternalInput")
    y_d = nc.dram_tensor("y", [T, C], BF16, kind="ExternalOutput")

    AF = mybir.ActivationFunctionType
    ALU = mybir.AluOpType

    with TileContext(nc) as tc:
        with (
            tc.tile_pool(name="const", bufs=1) as constp,
            tc.tile_pool(name="weights", bufs=1) as wp,
            tc.tile_pool(name="xin", bufs=1) as xp,
            tc.tile_pool(name="qk", bufs=1) as qkp,
            tc.tile_pool(name="vst", bufs=1) as vp,
            tc.tile_pool(name="pt", bufs=10) as ptp,
            tc.tile_pool(name="outT", bufs=1) as otp,
            tc.tile_pool(name="ystage", bufs=6) as ysp,
            tc.tile_pool(name="scps", bufs=2, space="PSUM") as scps,
            tc.tile_pool(name="avps", bufs=2, space="PSUM") as avps,
            tc.tile_pool(name="bcps", bufs=2, space="PSUM") as bcps,
        ):
            # --- weights + x chunk-0 arrive fused (one DMA per c-chunk) ---
            wxt = [
                wp.tile([128, GO + TQ], BF16, tag=f"wx{cc}", name=f"wx{cc}")
                for cc in range(NCC)
            ]
            xr = [
                xp.tile([128, T - TQ], BF16, tag=f"xr_{cc}", name=f"xr_{cc}")
                for cc in range(NCC)
            ]
            wq = [wxt[cc][:, 0:GO] for cc in range(NCC)]

            def xap(tch, cc):
                if tch == 0:
                    return wxt[cc][:, GO : GO + TQ]
                return xr[cc][:, (tch - 1) * TQ : tch * TQ]

            for cc in range(NCC):
                nc.sync.dma_start(
                    wxt[cc][:], wx_d[cc * 128 : (cc + 1) * 128, :]
                )

            # ---------------- constants ----------------
            bqk = constp.tile([128, 4], F32, tag="bqk")
            nc.sync.dma_start(bqk[:], bqk_d[:, :])
            bvb = constp.tile([128, HPC * D], F32, tag="bvb")
            nc.sync.dma_start(bvb[:], bvb_d[:, :])
            pad = constp.tile([128, NK], F32, tag="pad")
            nc.sync.dma_start(pad[:], pad_d[:, :])
            ones4 = constp.tile([128, HPC], BF16, tag="ones4")
            nc.vector.memset(ones4[:], 1.0)
            ones_row = constp.tile([65, 64], BF16, tag="ones_row")
            nc.vector.memset(ones_row[:], 1.0)
            recl_sb = [
                [
                    constp.tile([65, TQ], BF16, tag=f"recl{i}{h}", name=f"recl{i}{h}")
                    for h in range(2)
                ]
                for i in range(2)
            ]
            rec_sb = [
                [
                    constp.tile([65, TQ], F32, tag=f"rec{i}{h}", name=f"rec{i}{h}")
                    for h in range(2)
                ]
                for i in range(2)
            ]

            wo = []
            for j in range(2):
                w = wp.tile([128, C], BF16, tag=f"wo{j}", name=f"wo{j}")
                nc.sync.dma_start(w[:], woutT_d[j * 128 : (j + 1) * 128, :])
                wo.append(w)
            for cc in range(NCC):
                nc.sync.dma_start(
                    xr[cc][:], xT_d[cc * 128 : (cc + 1) * 128, TQ:T]
                )
            wo_odd = []
            for p in range(2):
                w = wp.tile([64, C], BF16, tag=f"wo_odd{p}")
                nc.sync.dma_start(
                    w[:], woutT_d[p * 128 + 64 : p * 128 + 128, :]
                )
                wo_odd.append(w)
            tmp_last = [
                [
                    otp.tile([64, TQ], BF16, tag=f"tl{p}{h}", name=f"tl{p}{h}")
                    for h in range(2)
                ]
                for p in range(2)
            ]

            # ---------------- static activation storage ----------------
            qt = [
                [
                    qkp.tile([128, TQ], BF16, tag=f"qt{p}_{i}", name=f"qt{p}_{i}")
                    for i in range(NQ)
                ]
                for p in range(2)
            ]
            kt = [
                [
                    qkp.tile([128, TQ], BF16, tag=f"kt{p}_{i}", name=f"kt{p}_{i}")
                    for i in range(NQ)
                ]
                for p in range(2)
            ]
            vt = [
                vp.tile([128, HPC, D + 1], BF16, tag=f"v{k}", name=f"v{k}")
                for k in range(NK)
            ]
            outT = [
                [
                    otp.tile([128, TQ], BF16, tag=f"o{p}_{q}", name=f"o{p}_{q}")
                    for q in range(NQ)
                ]
                for p in range(2)
            ]

            # ------- background PE blocks (QKV of next chunk, deferred proj) -
            def qkv_blocks(tch):
                blocks = []

                def mk_a(ot):
                    def blk():
                        pa = bcps.tile(
                            [128, TQ], F32, tag="bcyp", name=f"pa{tch}_{ot}"
                        )
                        for cc in range(NCC):
                            nc.tensor.matmul(
                                pa[:],
                                wq[cc][:, ot * 128 : (ot + 1) * 128],
                                xap(tch, cc),
                                start=(cc == 0),
                                stop=(cc == NCC - 1),
                            )
                        dst = qt[ot][tch] if ot < 2 else kt[ot - 2][tch]
                        nc.vector.tensor_scalar_add(
                            dst[:], pa[:], bqk[:, ot : ot + 1]
                        )

                    return blk

                def mk_b(tt):
                    def blk():
                        pb = bcps.tile(
                            [128, HPC * D], F32, tag="bcyp", name=f"pb{tch}_{tt}"
                        )
                        for cc in range(NCC):
                            nc.tensor.matmul(
                                pb[:],
                                xap(tch, cc)[:, tt * 128 : (tt + 1) * 128],
                                wq[cc][:, 2 * HPC * D : 3 * HPC * D],
                                start=(cc == 0),
                                stop=(cc == NCC - 1),
                            )
                        k_id = tch * 4 + tt
                        nc.vector.tensor_add(
                            vt[k_id][:, :, 0:D],
                            pb[:].rearrange("p (h d) -> p h d", d=D),
                            bvb[:].rearrange("p (h d) -> p h d", d=D),
                        )
                        nc.vector.tensor_copy(vt[k_id][:, :, D], ones4[:])

                    return blk

                for ot in range(4):
                    blocks.append(mk_a(ot))
                for tt in range(4):
                    blocks.append(mk_b(tt))
                return blocks

            def proj_blocks(qc, split_dma=False):
                blocks = []

                def mk(tt, oc):
                    def blk():
                        t0 = qc * TQ + tt * 128
                        ys = ysp.tile(
                            [128, TQ], BF16, tag="ys", name=f"ys{qc}{tt}{oc}"
                        )
                        yp = bcps.tile(
                            [128, TQ], F32, tag="bcyp", name=f"yp{qc}{tt}{oc}"
                        )
                        nc.tensor.matmul(
                            yp[:],
                            outT[0][qc][:, tt * 128 : (tt + 1) * 128],
                            wo[0][:, oc * TQ : (oc + 1) * TQ],
                            start=True,
                            stop=False,
                        )
                        nc.tensor.matmul(
                            yp[:],
                            outT[1][qc][:, tt * 128 : (tt + 1) * 128],
                            wo[1][:, oc * TQ : (oc + 1) * TQ],
                            start=False,
                            stop=True,
                        )
                        nc.vector.tensor_copy(ys[:], yp[:])
                        nc.sync.dma_start(
                            y_d[t0 : t0 + 128, oc * TQ : (oc + 1) * TQ], ys[:]
                        )

                    return blk

                for tt in range(4):
                    for oc in range(2):
                        blocks.append(mk(tt, oc))
                return blocks

            # ---------------- main schedule ----------------
            for blk in qkv_blocks(0):
                blk()

            for tch in range(NQ):
                qc = tch
                nk = (qc + 1) * 4

                bg = []
                if tch + 1 < NQ:
                    bg += qkv_blocks(tch + 1)
                # spread proj(qc-1) across the next two attention chunks
                if tch == 1:
                    bg += proj_blocks(0)[:2]
                elif tch == 2:
                    bg += proj_blocks(0)[2:] + proj_blocks(1)[:2]
                elif tch == 3:
                    bg += proj_blocks(1)[2:] + proj_blocks(2)
                nbg = len(bg)
                slots = 2 * nk
                emitted = 0
                slot = 0

                def inject():
                    nonlocal emitted
                    target = min(nbg, math.ceil(nbg * slot / slots))
                    while emitted < target:
                        bg[emitted]()
                        emitted += 1

                for p in ((1, 0) if qc >= NQ - 2 else (0, 1)):
                    av_e = avps.tile([65, TQ], F32, tag="av", name=f"ave{p}{qc}")
                    av_o = avps.tile([65, TQ], F32, tag="av", name=f"avo{p}{qc}")
                    pend = []  # (ki, pt) awaiting AV matmuls

                    def emit_av(ki, pt):
                        diag_j = ki - qc * 4 if ki >= qc * 4 else None
                        if diag_j is None:
                            regions = [(0, TQ, ki == 0, False)]
                        else:
                            v0 = 128 * diag_j
                            st = ki == 0
                            regions = [(v0, v0 + 128, st, True)]
                            if v0 + 128 < TQ:
                                regions.append((v0 + 128, TQ, st, False))
                        for h, av in ((0, av_e), (1, av_o)):
                            for c0, c1, st, sp in regions:
                                nc.tensor.matmul(
                                    av[:, c0:c1],
                                    vt[ki][:, 2 * p + h, :],
                                    pt[:, h, c0:c1],
                                    start=st,
                                    stop=sp,
                                    skip_group_check=True,
                                )

                    for ki in range(nk):
                        kch, kof = ki // 4, (ki % 4) * 128
                        diag_j = ki - qc * 4 if ki >= qc * 4 else None
                        vo = 128 * diag_j if diag_j is not None else 0
                        sc = scps.tile(
                            [128, 2 * TQ], F32, tag="sc", name=f"s{p}{qc}{ki}"
                        )
                        nc.tensor.matmul(
                            sc[:, vo:TQ],
                            kt[p][kch][0:64, kof : kof + KC],
                            qt[p][qc][0:64, vo:TQ],
                            start=True,
                            stop=True,
                        )
                        nc.tensor.matmul(
                            sc[:, TQ + vo : 2 * TQ],
                            kt[p][kch][64:128, kof : kof + KC],
                            qt[p][qc][64:128, vo:TQ],
                            start=True,
                            stop=True,
                        )
                        pt = ptp.tile([128, 2, TQ], BF16, tag="pt")
                        sc3 = sc[:].rearrange("p (h q) -> p h q", h=2)
                        if diag_j is None or diag_j == 0:
                            nc.scalar.activation(
                                pt[:], sc3[:], AF.Exp,
                                bias=pad[:, ki : ki + 1], scale=1.0,
                            )
                        else:
                            vw = TQ - vo
                            nc.scalar.activation(
                                pt[:, :, vo : vo + vw],
                                sc3[:, :, vo : vo + vw],
                                AF.Exp,
                                bias=pad[:, ki : ki + 1],
                                scale=1.0,
                            )
                        if diag_j is not None:
                            j = diag_j
                            q0 = 128 * j
                            for hh in range(2):
                                nc.gpsimd.affine_select(
                                    out=pt[:, hh, q0 : q0 + 128],
                                    in_=pt[:, hh, q0 : q0 + 128],
                                    compare_op=ALU.is_ge,
                                    fill=0.0,
                                    base=0,
                                    pattern=[[1, 128]],
                                    channel_multiplier=-1,
                                )
                        if len(pend) == 8:
                            emit_av(*pend.pop(0))
                            slot += 1
                            inject()
                        pend.append((ki, pt))
                    while pend:
                        emit_av(*pend.pop(0))
                        slot += 1
                        inject()

                    # finalize both heads with overlapped chains:
                    # recips first, then shift-DMAs, broadcasts, multiplies
                    # (odd head first -- its extra outT shift DMA is longest)
                    heads = ((1, av_o), (0, av_e))
                    if qc == NQ - 1 and p == 0:
                        # tail path: PE-matmul broadcast + staged AV copy,
                        # results land in partition-0-based tmp tiles that the
                        # final projection consumes directly (no shift DMA).
                        # The truly-last pair (p==0) stages the AV copy on the
                        # by-then-idle ACT engine; pair 1 uses DVE so it does
                        # not delay pair 0's exps queued on ACT.
                        for h, av in heads:
                            with nc.allow_low_precision(reason="softmax recip"):
                                nc.vector.reciprocal(
                                    recl_sb[p][h][64:65, :], av[64:65, :]
                                )
                        bcl = {}
                        for h, av in heads:
                            bc_ps = bcps.tile(
                                [64, TQ], F32, tag="bcyp", name=f"bcl{p}{h}"
                            )
                            nc.tensor.matmul(
                                bc_ps[:],
                                ones_row[64:65, :],
                                recl_sb[p][h][64:65, :],
                                start=True,
                                stop=True,
                            )
                            bcl[h] = bc_ps
                        avl = {}
                        for h, av in heads:
                            a = ysp.tile(
                                [64, TQ], BF16, tag="avsl", name=f"avsl{p}{h}"
                            )
                            if p == 0:
                                nc.scalar.copy(a[:], av[0:64, :])
                            else:
                                nc.vector.tensor_copy(a[:], av[0:64, :])
                            avl[h] = a
                        for h, av in heads:
                            nc.vector.tensor_mul(
                                tmp_last[p][h][:], avl[h][:], bcl[h][:]
                            )
                        continue
                    for h, av in heads:
                        with nc.allow_low_precision(reason="softmax recip"):
                            nc.vector.reciprocal(
                                rec_sb[p][h][64:65, :], av[64:65, :]
                            )
                    recbs = {}
                    for h, av in heads:
                        recb = ysp.tile(
                            [1, TQ], F32, tag="recb", name=f"recb{p}{qc}{h}"
                        )
                        nc.sync.dma_start(recb[:], rec_sb[p][h][64:65, :])
                        recbs[h] = recb
                    bcs = {}
                    for h, av in heads:
                        bc_sb = ysp.tile(
                            [64, TQ], F32, tag="bcsb", name=f"bcs{p}{qc}{h}"
                        )
                        nc.gpsimd.partition_broadcast(
                            bc_sb[:], recbs[h][:], channels=64
                        )
                        bcs[h] = bc_sb
                    for h, av in heads:
                        if h == 0:
                            nc.vector.tensor_mul(
                                outT[p][qc][0:64, :], av[0:64, :], bcs[h][:]
                            )
                        else:
                            tmp_o = ysp.tile(
                                [64, TQ], BF16, tag="tmpo", name=f"tmpo{p}{qc}"
                            )
                            nc.vector.tensor_mul(
                                tmp_o[:], av[0:64, :], bcs[h][:]
                            )
                            nc.sync.dma_start(
                                outT[p][qc][64:128, :], tmp_o[:]
                            )

                while emitted < nbg:
                    bg[emitted]()
                    emitted += 1

            # final projection: pair-1 outT plus the last pair's
            # partition-0 tmp tiles (split-K), psum->sbuf copies split
            # across DVE and ACT, per-half y DMAs
            qf = NQ - 1
            for tt in range(4):
                t0 = qf * TQ + tt * 128
                ys = ysp.tile([128, C], BF16, tag="ys", name=f"ysf{tt}")
                for oc in range(2):
                    yp = bcps.tile(
                        [128, TQ], F32, tag="bcyp", name=f"ypf{tt}{oc}"
                    )
                    nc.tensor.matmul(
                        yp[:],
                        outT[1][qf][:, tt * 128 : (tt + 1) * 128],
                        wo[1][:, oc * TQ : (oc + 1) * TQ],
                        start=True,
                        stop=False,
                    )
                    for i, hh in enumerate((0, 1)):
                        lhsT = tmp_last[0][hh][:, tt * 128 : (tt + 1) * 128]
                        rhs = (
                            wo[0][0:64, oc * TQ : (oc + 1) * TQ]
                            if hh == 0
                            else wo_odd[0][:, oc * TQ : (oc + 1) * TQ]
                        )
                        nc.tensor.matmul(
                            yp[:], lhsT, rhs,
                            start=False, stop=(i == 1),
                        )
                    if oc == 0:
                        nc.vector.tensor_copy(
                            ys[:, oc * TQ : (oc + 1) * TQ], yp[:]
                        )
                    else:
                        nc.scalar.copy(
                            ys[:, oc * TQ : (oc + 1) * TQ], yp[:]
                        )
                    nc.sync.dma_start(
                        y_d[t0 : t0 + 128, oc * TQ : (oc + 1) * TQ],
                        ys[:, oc * TQ : (oc + 1) * TQ],
                    )

    nc.compile()
    return nc


def _get_nc():
    global _CACHED_NC
    if _CACHED_NC is None:
        _CACHED_NC = _build()
    return _CACHED_NC


def _make_in_maps(x, attention_mask, W_qkv, b_qkv, W_out, b_out):
    bf16 = ml_dtypes.bfloat16
    x = np.asarray(x, dtype=np.float32)
    attention_mask = np.asarray(attention_mask, dtype=np.float32)
    W_qkv = np.asarray(W_qkv, dtype=np.float32)
    b_qkv = np.asarray(b_qkv, dtype=np.float32)
    W_out = np.asarray(W_out, dtype=np.float32)

    in_maps = []
    for core in range(NCORES):
        b = core // 4
        g = core % 4
        s = g * HPC * D
        e = (g + 1) * HPC * D
        Wq = W_qkv[s:e] * 0.125
        Wk = W_qkv[C + s : C + e]
        Wv = W_qkv[2 * C + s : 2 * C + e]
        xTb = x[b].T.astype(bf16)
        wqkvT = np.concatenate([Wq, Wk, Wv], axis=0).T.astype(bf16)
        wx = np.ascontiguousarray(np.concatenate([wqkvT, xTb[:, 0:TQ]], axis=1))
        woutT = np.ascontiguousarray(W_out[:, s:e].T.astype(bf16))
        bq = b_qkv[s:e] * 0.125
        bk = b_qkv[C + s : C + e]
        bqk = np.ascontiguousarray(
            np.stack([bq[0:128], bq[128:256], bk[0:128], bk[128:256]], axis=1)
        )
        bv = b_qkv[2 * C + s : 2 * C + e]
        bvb = np.ascontiguousarray(np.broadcast_to(bv, (128, HPC * D)))
        padv = np.ascontiguousarray(
            ((1.0 - attention_mask[b]) * NEG).reshape(NK, 128).T
        )
        in_maps.append(
            {
                "xT": np.ascontiguousarray(xTb),
                "wx": wx,
                "woutT": woutT,
                "bqk": bqk,
                "bvb": bvb,
                "pad": padv,
            }
        )
    return in_maps


def kernel(x, attention_mask, W_qkv, b_qkv, W_out, b_out, _trace=False):
    nc = _get_nc()
    in_maps = _make_in_maps(x, attention_mask, W_qkv, b_qkv, W_out, b_out)
    res = run_bass_kernel_spmd(
        nc, in_maps, core_ids=list(range(NCORES)), trace=_trace
    )
    B = np.asarray(x).shape[0]
    y = np.zeros((B, T, C), dtype=np.float32)
    for b in range(B):
        acc = res.results[4 * b]["y"].astype(np.float32)
        for g in range(1, 4):
            acc = acc + res.results[4 * b + g]["y"].astype(np.float32)
        y[b] = acc
    y += np.asarray(b_out, dtype=np.float32)
    if _trace:
        kernel._last_results = res
    return y
